# revision 12
# baseline (speedup 1.0000x reference)
"""Trainium2 Bass kernel for nn_CADense (context-adaptive low-rank dense layer).

Computes, for the full batch:
    s_mod = s + context @ w          # [B, R]
    low   = (data @ u) * s_mod       # [B, R]
    out   = relu(low @ v.T + 2*bias) # [B, UNITS]

Sharding: data-parallel over batch across 8 NeuronCores; u/s/v/w/bias
replicated. Each core runs the same Bass program on its 1024-row shard.

The PE contracts over the partition dim, so the big operands are marshaled
host-side into contraction-major layouts (data.T, context.T, v.T) when the
shards are built — on-chip PE transposes would otherwise dominate the
kernel. All matmuls run as float32r (full-rate fp32 streaming mode).

Compute is done in the "transposed" domain per rank-chunk:
    lowT[r, b] = (u.T @ data.T)[r, b] * (s[r] + (w.T @ ctx.T)[r, b])
with the s-add fused into the scalar-engine PSUM evacuation. The final
matmul returns to natural [b, units] layout; the 2*bias add is folded in
as a K=1 rank-1 matmul into the same PSUM accumulation group and ReLU
evacuation of the output PSUM groups alternates between the scalar and
vector engines so neither gates PSUM recycling.

Schedule notes:
- Input DMAs are spread across both HWDGE queues (sync: data tiles,
  scalar: weights/context) and output stores go through the gpsimd SWDGE
  queue — three independent descriptor rings so transfers overlap and
  the HBM link stays saturated.
- The two 512-row batch tiles are software-pipelined; PE emission
  interleaves batch-tile 1's rank stage with batch-tile 0's output stage
  and the (DMA-independent) context matmuls fill data-DMA wait bubbles,
  so the PE never idles long enough for the HAM clock gate to
  re-throttle.
- A short burst of bf16 dummy matmuls on garbage SBUF pre-warms the HAM
  clock gate while the first DMAs stream in.
"""

import os
import sys
from contextlib import ExitStack

import numpy as np


def _ensure_concourse():
    try:
        import concourse  # noqa: F401
    except ImportError:
        for p in ("/opt/trn_rl_repo", "/root/.axon_site/_ro/trn_rl_repo"):
            if os.path.isdir(p) and p not in sys.path:
                sys.path.insert(0, p)


_ensure_concourse()

import concourse.tile as tile  # noqa: E402
from concourse import bacc, mybir  # noqa: E402
from concourse.bass_utils import run_bass_kernel_spmd  # noqa: E402

NCORES = 8
B, N_IN, UNITS, RANK, CCTX = 8192, 2048, 2048, 256, 512
NB = B // NCORES  # batch rows per core
P = 128
BT = 512  # batch tile (free dim of T-domain matmuls)
NBT = NB // BT  # batch tiles per core
KC = N_IN // P  # 16 contraction chunks for data @ u
CC = CCTX // P  # 4 contraction chunks for context @ w
RC = RANK // P  # 2 rank chunks
MS = 512  # output units slice width
NMS = UNITS // MS  # 4 unit slices
N_WARMUP_MM = 14

F32 = mybir.dt.float32
F32R = mybir.dt.float32r
BF16 = mybir.dt.bfloat16


def _emit(nc, tc, ctx):
    # Host-marshaled transposed layouts: dataT = data.T, ctxT = context.T,
    # vT = v.T (built per-shard in kernel()).
    d_dataT = nc.dram_tensor("dataT", [N_IN, NB], F32R, kind="ExternalInput")
    d_ctxT = nc.dram_tensor("ctxT", [CCTX, NB], F32R, kind="ExternalInput")
    d_u = nc.dram_tensor("u", [N_IN, RANK], F32R, kind="ExternalInput")
    d_s = nc.dram_tensor("s", [RANK], F32, kind="ExternalInput")
    d_vT = nc.dram_tensor("vT", [RANK, UNITS], F32R, kind="ExternalInput")
    d_w = nc.dram_tensor("w", [CCTX, RANK], F32R, kind="ExternalInput")
    d_bias = nc.dram_tensor("bias", [UNITS], F32R, kind="ExternalInput")
    d_out = nc.dram_tensor("out", [NB, UNITS], F32, kind="ExternalOutput")

    ap_dataT = d_dataT.ap().rearrange("(q j p) b -> p q j b", p=P, j=4)
    ap_ctxT = d_ctxT.ap().rearrange("(cc p) b -> p cc b", p=P)
    ap_u = d_u.ap().rearrange("(uq j p) r -> p uq j r", p=P, j=4)
    ap_vT = d_vT.ap().rearrange("(rc p) m -> p rc m", p=P)

    singles = ctx.enter_context(tc.tile_pool(name="singles", bufs=1))
    du_psum = ctx.enter_context(tc.tile_pool(name="du_psum", bufs=2, space="PSUM"))
    s_psum = ctx.enter_context(tc.tile_pool(name="s_psum", bufs=2, space="PSUM"))
    o_psum = ctx.enter_context(tc.tile_pool(name="o_psum", bufs=4, space="PSUM"))
    dTpool = ctx.enter_context(tc.tile_pool(name="dataT", bufs=1))
    cTpool = ctx.enter_context(tc.tile_pool(name="ctxT", bufs=2))
    lowpool = ctx.enter_context(tc.tile_pool(name="lowT", bufs=2))
    smodpool = ctx.enter_context(tc.tile_pool(name="smod", bufs=4))
    opool = ctx.enter_context(tc.tile_pool(name="outsb", bufs=3))

    # HAM warm-up fodder: garbage bf16 matmuls while the first loads stream.
    wu_a = singles.tile([P, P], BF16)
    nc.vector.memset(wu_a[:], 1.0)
    wu_b = singles.tile([P, MS], BF16)
    nc.vector.memset(wu_b[:], 1.0)

    # ---- input DMA queues, in first-use order --------------------------
    # sync HWDGE ring: the eight 1 MiB dataT quads.
    dataT_t = {0: [], 1: []}
    for bt in range(NBT):
        for q4 in range(4):
            q = dTpool.tile(
                [P, 4, BT], F32R, tag=f"dataT{bt}q{q4}", name=f"dataT{bt}q{q4}"
            )
            nc.sync.dma_start(
                out=q[:], in_=ap_dataT[:, q4, :, bt * BT : (bt + 1) * BT]
            )
            dataT_t[bt] += [q[:, j] for j in range(4)]

    # scalar HWDGE ring: weights and context.
    u_t = []  # u_t[uq] = [P, 4, RANK] tile; chunk kc = u_t[kc//4][:, kc%4]
    uq0 = singles.tile([P, 4, RANK], F32R, name="uq0")
    nc.scalar.dma_start(out=uq0[:], in_=ap_u[:, 0])
    u_t.append(uq0)
    w_sb = singles.tile([P, CC, RANK], F32R)
    nc.scalar.dma_start(
        out=w_sb[:], in_=d_w.ap().rearrange("(cc p) r -> p cc r", p=P)
    )
    for uq in (1, 2, 3):
        ut = singles.tile([P, 4, RANK], F32R, name=f"uq{uq}")
        nc.scalar.dma_start(out=ut[:], in_=ap_u[:, uq])
        u_t.append(ut)
    ctxT_t = {}
    ctxT_t[0] = cTpool.tile([P, CC, BT], F32R, tag="ctxT", name="ctxT0")
    nc.scalar.dma_start(out=ctxT_t[0][:], in_=ap_ctxT[:, :, 0:BT])
    s_sb = singles.tile([P, RC], F32)
    nc.scalar.dma_start(out=s_sb[:], in_=d_s.ap().rearrange("(rc p) -> p rc", p=P))
    bias2 = singles.tile([1, UNITS], F32R)
    nc.scalar.dma_start(out=bias2[:], in_=d_bias.ap().rearrange("(a m) -> a m", a=1))
    vT_sb = singles.tile([P, RC, UNITS], F32R)
    nc.scalar.dma_start(out=vT_sb[:, 0], in_=ap_vT[:, 0])
    nc.scalar.dma_start(out=vT_sb[:, 1], in_=ap_vT[:, 1])
    ctxT_t[1] = cTpool.tile([P, CC, BT], F32R, tag="ctxT", name="ctxT1")
    nc.scalar.dma_start(out=ctxT_t[1][:], in_=ap_ctxT[:, :, BT:])

    ones_f = singles.tile([1, P], F32)
    nc.vector.memset(ones_f[:], 2.0)
    ones = singles.tile([1, P], F32R)
    nc.vector.tensor_copy(out=ones[:], in_=ones_f[:])

    # ---- HAM warm-up ---------------------------------------------------
    wu_ps = o_psum.tile([P, MS], F32, tag="po", name="wu_ps")
    for _ in range(N_WARMUP_MM):
        nc.tensor.matmul(wu_ps[:], lhsT=wu_a[:], rhs=wu_b[:], start=True, stop=True)

    # ---- compute stages ------------------------------------------------
    lowT_t = {}
    pd_t = {}
    smod_t = {}

    def emit_rank_mms(bt, kc_lo, kc_hi):
        """mm1T k-chunks [kc_lo, kc_hi) for both rank chunks."""
        if kc_lo == 0:
            pd_t[bt] = [
                du_psum.tile([P, BT], F32, tag="pd", name="pd") for _ in range(RC)
            ]
        for kc in range(kc_lo, kc_hi):
            for rc in range(RC):
                nc.tensor.matmul(
                    pd_t[bt][rc][:],
                    lhsT=u_t[kc // 4][:, kc % 4, rc * P : (rc + 1) * P],
                    rhs=dataT_t[bt][kc],
                    start=(kc == 0),
                    stop=(kc == KC - 1),
                )

    def emit_smod(bt):
        """ctx @ w matmuls + s-add; independent of the data stream."""
        smod_t[bt] = []
        for rc in range(RC):
            ps = s_psum.tile([P, BT], F32, tag="ps", name="ps")
            for cc in range(CC):
                nc.tensor.matmul(
                    ps[:],
                    lhsT=w_sb[:, cc, rc * P : (rc + 1) * P],
                    rhs=ctxT_t[bt][:, cc, :],
                    start=(cc == 0),
                    stop=(cc == CC - 1),
                )
            smod = smodpool.tile([P, BT], F32, tag="smod", name="smod")
            nc.scalar.add(smod[:], ps[:], add=s_sb[:, rc : rc + 1])
            smod_t[bt].append(smod)

    def emit_mul(bt):
        """lowT = pd * smod on the vector engine."""
        lowT_t[bt] = lowpool.tile([P, RC, BT], F32R, tag="lowT", name="lowT")
        for rc in range(RC):
            nc.vector.tensor_mul(
                out=lowT_t[bt][:, rc, :], in0=pd_t[bt][rc][:], in1=smod_t[bt][rc][:]
            )

    def emit_out_stage(bt, bc, fine_stores=False):
        """out[b, :] = relu(low @ v.T + 2*bias) for one 128-row chunk.

        All four 512-wide PSUM groups stay open at once and the matmuls
        are ordered rc-major so consecutive matmuls reuse the same
        stationary operand; ReLU evacuation alternates between the
        scalar and vector engines.
        """
        b0 = bt * BT
        lowT = lowT_t[bt]
        osb = opool.tile([P, UNITS], F32, tag="osb", name="osb")
        pos = [o_psum.tile([P, MS], F32, tag="po", name="po") for _ in range(NMS)]
        for rc in range(RC):
            for ms in range(NMS):
                nc.tensor.matmul(
                    pos[ms][:],
                    lhsT=lowT[:, rc, bc * P : (bc + 1) * P],
                    rhs=vT_sb[:, rc, ms * MS : (ms + 1) * MS],
                    start=(rc == 0),
                    stop=False,
                )
        for ms in range(NMS):
            nc.tensor.matmul(
                pos[ms][:],
                lhsT=ones[:],
                rhs=bias2[:, ms * MS : (ms + 1) * MS],
                start=False,
                stop=True,
            )
        rows = slice(b0 + bc * P, b0 + (bc + 1) * P)
        for ms in range(NMS):
            sl = slice(ms * MS, (ms + 1) * MS)
            if ms % 2 == 0:
                nc.scalar.activation(
                    osb[:, sl], pos[ms][:], mybir.ActivationFunctionType.Relu
                )
            else:
                nc.vector.tensor_relu(out=osb[:, sl], in_=pos[ms][:])
            if fine_stores:
                nc.gpsimd.dma_start(out=d_out.ap()[rows, sl], in_=osb[:, sl])
        if not fine_stores:
            nc.gpsimd.dma_start(out=d_out.ap()[rows, :], in_=osb[:])

    # Software pipeline across the two batch tiles.
    emit_rank_mms(0, 0, KC)
    emit_smod(0)
    emit_mul(0)
    emit_out_stage(0, 0)
    emit_out_stage(0, 1)
    emit_rank_mms(1, 0, 8)
    emit_out_stage(0, 2)
    emit_smod(1)
    emit_rank_mms(1, 8, KC)
    emit_out_stage(0, 3)
    emit_mul(1)
    for bc in range(BT // P):
        emit_out_stage(1, bc, fine_stores=(bc == BT // P - 1))


_CACHE = {}


def build():
    if "nc" in _CACHE:
        return _CACHE["nc"]
    nc = bacc.Bacc("TRN2", target_bir_lowering=False, debug=False)
    with tile.TileContext(nc) as tc, ExitStack() as ctx:
        _emit(nc, tc, ctx)
    nc.compile()
    _CACHE["nc"] = nc
    return nc


def make_in_maps(data, context, u, s, v, w, bias):
    u = np.ascontiguousarray(np.asarray(u, dtype=np.float32))
    s = np.ascontiguousarray(np.asarray(s, dtype=np.float32))
    vT = np.ascontiguousarray(np.asarray(v, dtype=np.float32).T)
    w = np.ascontiguousarray(np.asarray(w, dtype=np.float32))
    bias = np.ascontiguousarray(np.asarray(bias, dtype=np.float32))
    in_maps = []
    for c in range(NCORES):
        sl = slice(c * NB, (c + 1) * NB)
        in_maps.append(
            {
                "dataT": np.ascontiguousarray(np.asarray(data[sl], dtype=np.float32).T),
                "ctxT": np.ascontiguousarray(
                    np.asarray(context[sl], dtype=np.float32).T
                ),
                "u": u,
                "s": s,
                "vT": vT,
                "w": w,
                "bias": bias,
            }
        )
    return in_maps


def kernel(data, context, u, s, v, w, bias):
    nc = build()
    in_maps = make_in_maps(data, context, u, s, v, w, bias)
    res = run_bass_kernel_spmd(nc, in_maps, core_ids=list(range(NCORES)))
    return np.concatenate([r["out"] for r in res.results], axis=0)
